# revision 1
# baseline (speedup 1.0000x reference)
"""GAT layer kernel for Trainium2, 8 NeuronCores (SPMD) — v2.

Strategy (edge-sharded by dst, node table replicated):
  Host: sort edges by dst; dst-tiles (128 nodes) assigned to cores in
  contiguous ranges (49 slots/core). Edge indices/layout precomputed.
  Phase A (each core, replicated): z = h @ W_lin.T plus per-node score
    scalars a_src/a_dst; writes a DRAM table row per node:
    [z | 1.0 | a_src f32 | a_dst f32] (256-elem rows for dma_gather).
  Phase B per dst-tile slot:
    - dma_gather table rows by edge src (int16 idx, lo/hi halves, 8 queues)
    - adT_row: strided DMA of the slot's 128 a_dst values -> broadcast
      across partitions via a K=1 PE matmul -> adT_rep [128,128]
    - edge scores B[p,c,j] = a_src[p,c] + adT_rep[p,j] (one DVE bcast add)
      -> W = exp(leaky_relu(B)) via two Activation passes (Lrelu alpha=.01)
    - per chunk: S_w = (iota==dl)*W  (one fused scalar_tensor_tensor)
      and ONE matmul ps[:,0:129] += S_w^T @ [z | 1] accumulating numerator
      and denominator together.
    - out tile = ps[:,0:128] * reciprocal(ps[:,128])
Host gathers per-core output slots back into the full [N, 128] array.
"""

import os
import sys
import types
import numpy as np

N = 50000
E = 1600000
D = 128
P = 128
NTILES = (N + P - 1) // P            # 391
NNODES_PAD = NTILES * P              # 50048
TSLOTS = 49                          # tile-slots per core (8*49 >= 391)
SPLIT = 32768                        # int16 gather index limit
NEG_SLOPE = 0.01
PAD_DSTLOC = 200.0                   # sentinel: never matches iota 0..127
MAX_GIDX = 512                       # dma_gather idx limit (scratch/4 queues)
NQUEUES = 4

ROW_FP8 = False                      # 256B fp8e3 rows vs 512B bf16 rows

LAST_EXEC_NS = None

# ---------------------------------------------------------------- toolchain fixes


def _apply_tilefix():
    import concourse.tile as tile_mod
    from concourse._compat import not_none as nn
    from concourse.vector_clock import ScopedClock

    def _patched_drain_and_barrier(self, tick_clock, wait_clock):
        nc = self.nc
        probe = nc.sync.nop()
        wait_clock.add_sem_waits(
            probe.ins, ScopedClock({None: tick_clock.global_clock}))
        si = probe.ins.sync_info
        waits = list(si.on_wait) if si is not None and si.on_wait else []
        nn(nc.cur_bb).bb.instructions.remove(probe.ins)
        by_name = {h.name: h for h in self.sems.allocated().values()}
        for w in waits:
            h = by_name[w.ant_name]
            assert w.wait_mode == "sem-ge-imm", w.wait_mode
            nc.sync.wait_ge(h, w.wait_value)
        nc.sync.drain()
        nc.all_engine_barrier()
        assert self.sems is not None
        popped = nc._tile_sem_poison_stack.pop()
        assert popped is self._sem_poison
        nc.clear_and_free_semaphores(list(self.sems.allocated().values()))
        nc.all_engine_barrier()

    tile_mod.TileContext._drain_and_barrier = _patched_drain_and_barrier


def _legalize_waits(nc):
    """This container's walrus caps sync waits at 1 per instruction; hoist
    extras onto standalone EventSemaphore (wait) instructions."""
    import concourse.mybir as mybir
    MAXW = 1
    for f in nc.m.functions:
        for bb in f.blocks:
            insts = bb.instructions
            new_list = []
            changed = False
            for ins in list(insts):
                si = ins.sync_info
                waits = list(si.on_wait) if (si is not None and si.on_wait) else []
                if len(waits) > MAXW:
                    changed = True
                    extra, keep = waits[:-MAXW], waits[-MAXW:]
                    for j in range(0, len(extra), MAXW):
                        chunk = extra[j:j + MAXW]
                        ev = mybir.InstEventSemaphore(
                            name=f"{ins.name}-waitfix{j}", ins=[], outs=[])
                        ev.engine = ins.engine
                        ev.sync_info = mybir.SyncInfo(on_wait=chunk, on_update=[])
                        new_list.append(ev)
                    si.on_wait = keep
                new_list.append(ins)
            if changed:
                bb.instructions = new_list


def _apply_profhook():
    try:
        import antenv.axon_hooks  # noqa: F401
        return
    except ImportError:
        pass
    try:
        from trn_agent_boot.trn_boot import _ntff_profile_via_ctypes
        hook = _ntff_profile_via_ctypes('/opt/axon/libaxon_pjrt.so')
    except Exception:
        hook = None
    mod = types.ModuleType('antenv.axon_hooks')
    mod._hook = hook
    mod.get_axon_ntff_profile_hook = lambda: mod._hook
    mod.set_axon_ntff_profile_hook = lambda h: setattr(mod, '_hook', h)
    sys.modules['antenv.axon_hooks'] = mod


# ---------------------------------------------------------------- host prep


def _ceil_div(a, b):
    return -(-a // b)


def _wrap_idx(arr):
    """[n] int16 -> [128, n/16] wrapped-in-16-partitions, replicated x8."""
    a = np.asarray(arr, dtype=np.int16)
    assert a.size % 16 == 0
    w = a.reshape(-1, 16).T.copy()            # [16, n/16]
    return np.tile(w, (8, 1))                 # [128, n/16]


def _prepare(src, dst):
    """Sort edges by dst; build per-core, per-slot chunk schedules and index
    arrays. Returns (schedule, per_core_data)."""
    src = np.asarray(src).astype(np.int64)
    dst = np.asarray(dst).astype(np.int64)
    order = np.argsort(dst, kind="stable")
    s_s = src[order]
    d_s = dst[order]

    tile_start = np.searchsorted(d_s, np.arange(0, NNODES_PAD + P, P))
    n_lo = np.zeros((8, TSLOTS), np.int64)
    n_hi = np.zeros((8, TSLOTS), np.int64)
    edges_lo = {}
    edges_hi = {}
    for m in range(8):
        for k in range(TSLOTS):
            t = m * TSLOTS + k
            if t >= NTILES:
                continue
            e0, e1 = tile_start[t], tile_start[t + 1]
            ss = s_s[e0:e1]
            dl = (d_s[e0:e1] - t * P)
            lo = ss < SPLIT
            sl, dll = ss[lo], dl[lo]
            o = np.argsort(sl, kind="stable")
            edges_lo[(m, k)] = (sl[o], dll[o])
            sh, dlh = ss[~lo] - SPLIT, dl[~lo]
            o = np.argsort(sh, kind="stable")
            edges_hi[(m, k)] = (sh[o], dlh[o])
            n_lo[m, k] = int(lo.sum())
            n_hi[m, k] = int((~lo).sum())

    C_lo = [max(1, _ceil_div(int(n_lo[:, k].max()), P)) for k in range(TSLOTS)]
    C_hi = [max(1, _ceil_div(int(n_hi[:, k].max()), P)) for k in range(TSLOTS)]

    per_core = []
    for m in range(8):
        idx_cols = []      # int16 wrapped cols, concatenated along axis 1
        dstloc_cols = []   # [P, C] f32 per slot
        for k in range(TSLOTS):
            parts = []
            for (edges, C) in ((edges_lo.get((m, k), (np.zeros(0, np.int64),) * 2), C_lo[k]),
                               (edges_hi.get((m, k), (np.zeros(0, np.int64),) * 2), C_hi[k])):
                ss, dl = edges
                n = C * P
                idx = np.zeros(n, np.int64)
                dlc = np.full(n, PAD_DSTLOC, np.float32)
                idx[: ss.size] = ss
                dlc[: dl.size] = dl.astype(np.float32)
                parts.append((idx, dlc, C))
            slot_dl = []
            for idx, dlc, C in parts:
                j = 0
                while j < idx.size:
                    n_sub = min(MAX_GIDX, idx.size - j)
                    idx_cols.append(_wrap_idx(idx[j:j + n_sub]))
                    j += n_sub
                slot_dl.append(dlc.reshape(C, P).T)     # [P, C]
            dstloc_cols.append(np.concatenate(slot_dl, axis=1))
        idx_all = np.concatenate(idx_cols, axis=1).astype(np.int16)  # [P, sum]
        dstloc_all = np.concatenate(dstloc_cols, axis=1).astype(np.float32)
        nid = np.zeros((P, TSLOTS), np.int32)
        for k in range(TSLOTS):
            t = m * TSLOTS + k
            if t < NTILES:
                nid[:, k] = np.arange(t * P, t * P + P, dtype=np.int32)
        per_core.append({"idx": idx_all, "dstloc": dstloc_all, "nid": nid})

    schedule = (tuple(C_lo), tuple(C_hi))
    return schedule, per_core


# ---------------------------------------------------------------- device program

_BUILD_CACHE = {}

# row layout (in row-dtype elements)
RLEN = 256
ONE_OFF = 128      # constant 1.0


def _build(schedule, idx_width):
    import concourse.bass as bass
    import concourse.mybir as mybir
    import concourse.tile as tile
    from concourse import bacc, library_config

    C_lo, C_hi = schedule
    C_tot = [a + b for a, b in zip(C_lo, C_hi)]

    nc = bacc.Bacc("TRN2", dynamic_dma_scratch_size=131072,
                   num_swdge_queues=NQUEUES)
    f32 = mybir.dt.float32
    i16 = mybir.dt.int16
    bf16 = mybir.dt.bfloat16
    row_dt = mybir.dt.float8e3 if ROW_FP8 else bf16
    row_b = 1 if ROW_FP8 else 2            # bytes per row elem
    ASRC_ELEM = 130                        # a_src bf16 (2 bytes)
    ADST_ELEM = 130 + 2 // row_b           # a_dst bf16 (2 bytes)
    ROW_W = ADST_ELEM + 2 // row_b         # row elems written

    hT = nc.dram_tensor("hT", [P, NNODES_PAD], bf16, kind="ExternalInput")
    embT = nc.dram_tensor("embT", [P, NNODES_PAD], bf16, kind="ExternalInput")
    W_lin = nc.dram_tensor("W_lin", [P, P], bf16, kind="ExternalInput")
    W_linT = nc.dram_tensor("W_linT", [P, P], bf16, kind="ExternalInput")
    wfc = nc.dram_tensor("wfc", [P, 2], bf16, kind="ExternalInput")
    wemb = nc.dram_tensor("wemb", [P, 2], bf16, kind="ExternalInput")
    iota_in = nc.dram_tensor("iota_in", [P, P], bf16, kind="ExternalInput")
    ident_in = nc.dram_tensor("ident_in", [P, P], bf16, kind="ExternalInput")
    nid_in = nc.dram_tensor("nid_in", [P, TSLOTS], mybir.dt.int32,
                            kind="ExternalInput")
    idx_in = nc.dram_tensor("idx_in", [P, idx_width], i16, kind="ExternalInput")
    dstloc_in = nc.dram_tensor("dstloc_in", [P, sum(C_tot)], f32,
                               kind="ExternalInput")

    table = nc.dram_tensor("table", [NNODES_PAD, RLEN], row_dt)
    out_d = nc.dram_tensor("out", [TSLOTS * P, P], f32, kind="ExternalOutput")

    with tile.TileContext(nc) as tc:
        with tc.tile_pool(name="const", bufs=1) as cpool:
            nc.gpsimd.load_library(library_config.mlp)
            iota_t = cpool.tile([P, P], bf16)
            ident_t = cpool.tile([P, P], bf16)
            rh1 = cpool.tile([P, 130], bf16)       # [W_linT | u1 | u2]
            wemb_t = cpool.tile([P, 2], bf16)
            wl_t = cpool.tile([P, P], bf16)
            wfc_t = cpool.tile([P, 2], bf16)
            ones_row = cpool.tile([1, P], bf16)
            nc.sync.dma_start(out=iota_t[:], in_=iota_in[:])
            nc.sync.dma_start(out=ident_t[:], in_=ident_in[:])
            nc.sync.dma_start(out=rh1[:, 0:P], in_=W_linT[:])
            nc.sync.dma_start(out=wemb_t[:], in_=wemb[:])
            nc.sync.dma_start(out=wl_t[:], in_=W_lin[:])
            nc.sync.dma_start(out=wfc_t[:], in_=wfc[:])
            nc.vector.memset(ones_row[:], 1.0)

            # ---- u1/u2 = W_lin.T @ w_fc halves -> rh1[:, 128:130]
            with tc.tile_pool(name="upsum", bufs=1, space="PSUM") as upp:
                ups = upp.tile([P, 2], f32, space="PSUM")
                nc.tensor.matmul(ups[:], lhsT=wl_t[:], rhs=wfc_t[:],
                                 start=True, stop=True)
                nc.vector.tensor_copy(out=rh1[:, P:P + 2], in_=ups[:])

            # ---- Phase A: build node table (batched DMA: 16-tile loads,
            # 4-tile stores -- phase A is sync-engine-issue bound otherwise)
            LCH = 16
            SCH = 4
            with tc.tile_pool(name="pa", bufs=2) as pa, \
                 tc.tile_pool(name="pas", bufs=3) as pas, \
                 tc.tile_pool(name="pap", bufs=4, space="PSUM") as pap:
                for t0 in range(0, NTILES, LCH):
                    nt = min(LCH, NTILES - t0)
                    hch = pa.tile([P, LCH * P], bf16, tag="hch")
                    ech = pa.tile([P, LCH * P], bf16, tag="ech")
                    nc.sync.dma_start(
                        out=hch[:, 0:nt * P], in_=hT[:, t0 * P:(t0 + nt) * P])
                    nc.sync.dma_start(
                        out=ech[:, 0:nt * P], in_=embT[:, t0 * P:(t0 + nt) * P])
                    for s0 in range(0, nt, SCH):
                        ns = min(SCH, nt - s0)
                        row4 = pas.tile([P, SCH, ROW_W], row_dt, tag="row4")
                        for i in range(ns):
                            t = t0 + s0 + i
                            lh = hch[:, (s0 + i) * P:(s0 + i + 1) * P]
                            le = ech[:, (s0 + i) * P:(s0 + i + 1) * P]
                            ps_a = pap.tile([P, 130], f32, space="PSUM",
                                            tag="ps_a")
                            nc.tensor.matmul(ps_a[:], lhsT=lh, rhs=rh1[:],
                                             start=True, stop=False)
                            nc.tensor.matmul(ps_a[:, P:P + 2], lhsT=le,
                                             rhs=wemb_t[:],
                                             start=False, stop=True)
                            rw = row4[:, i, :]
                            nc.scalar.copy(out=rw[:, 0:P], in_=ps_a[:, 0:P])
                            nc.vector.memset(rw[:, ONE_OFF:ONE_OFF + 1], 1.0)
                            nc.vector.tensor_copy(
                                out=rw[:, ASRC_ELEM:ASRC_ELEM + 4 // row_b]
                                .bitcast(bf16),
                                in_=ps_a[:, P:P + 2])
                        tt = t0 + s0
                        nc.sync.dma_start(
                            out=table[tt * P:(tt + ns) * P, 0:ROW_W]
                            .rearrange("(s p) e -> p s e", s=ns),
                            in_=row4[:, 0:ns, :])

            # ---- Phase B: per dst-tile slot
            with tc.tile_pool(name="pb", bufs=3) as pb, \
                 tc.tile_pool(name="pbs", bufs=3) as pbs, \
                 tc.tile_pool(name="pbw", bufs=2) as pbw, \
                 tc.tile_pool(name="pbp", bufs=2, space="PSUM") as pbp, \
                 tc.tile_pool(name="pbp2", bufs=2, space="PSUM") as pbp2:
                idx_off = 0
                dl_off = 0
                gq = 0
                nida = pbs.tile([P, TSLOTS], mybir.dt.int32, tag="nida")
                nc.sync.dma_start(out=nida[:], in_=nid_in[:])
                for k in range(TSLOTS):
                    C = C_tot[k]
                    gbuf = pb.tile([P, C, RLEN], row_dt, tag="gbuf")
                    wk = (C_lo[k] + C_hi[k]) * P // 16
                    it = pbs.tile([P, wk], i16, tag="idx")
                    nc.sync.dma_start(
                        out=it[:], in_=idx_in[:, idx_off:idx_off + wk])
                    idx_off += wk
                    iw = 0
                    for (Ch, base) in ((C_lo[k], 0), (C_hi[k], SPLIT)):
                        ntot = Ch * P
                        cpos = 0 if base == 0 else C_lo[k]
                        j = 0
                        while j < ntot:
                            n_sub = min(MAX_GIDX, ntot - j)
                            w16 = n_sub // 16
                            c0 = cpos + j // P
                            nsc = n_sub // P
                            if base == 0:
                                src_ap = table[0:SPLIT]
                            else:
                                src_ap = table[SPLIT:NNODES_PAD]
                            nc.gpsimd.dma_gather(
                                out_ap=gbuf[:, c0:c0 + nsc, :], in_ap=src_ap,
                                idxs_ap=it[:, iw:iw + w16], num_idxs=n_sub,
                                num_idxs_reg=n_sub, elem_size=RLEN,
                                queue_num=gq % NQUEUES)
                            gq += 1
                            iw += w16
                            j += n_sub

                    # slot's a_dst values via indirect fetch -> transpose
                    # -> bcast across partitions
                    ad_bt = pbs.tile([P, 2 // row_b], row_dt, tag="ad_bt")
                    nc.gpsimd.indirect_dma_start(
                        out=ad_bt[:], out_offset=None,
                        in_=table[:],
                        in_offset=bass.IndirectOffsetOnAxis(
                            ap=nida[:, k:k + 1], axis=0),
                        element_offset=ADST_ELEM)
                    ad_col = ad_bt[:].bitcast(bf16)
                    ps_aT = pbp2.tile([1, P], f32, space="PSUM", tag="ps_aT")
                    nc.tensor.matmul(ps_aT[:], lhsT=ad_col, rhs=ident_t[:],
                                     start=True, stop=True)
                    adT_row = pbs.tile([1, P], bf16, tag="adT_row")
                    nc.scalar.copy(out=adT_row[:], in_=ps_aT[:])
                    ps_ar = pbp2.tile([P, P], f32, space="PSUM", tag="ps_ar")
                    nc.tensor.matmul(ps_ar[:], lhsT=ones_row[:], rhs=adT_row[:],
                                     start=True, stop=True)
                    adT_rep = pbs.tile([P, P], bf16, tag="adT_rep")
                    nc.scalar.copy(out=adT_rep[:], in_=ps_ar[:])

                    # dst_local columns for this slot
                    dl_t = pbs.tile([P, C], f32, tag="dl_t")
                    nc.sync.dma_start(
                        out=dl_t[:], in_=dstloc_in[:, dl_off:dl_off + C])
                    dl_off += C

                    # W[p,c,j] = exp(lrelu(a_src[p,c] + adT_rep[p,j]))
                    w_t = pbw.tile([P, C, P], bf16, tag="w_t")
                    asrc_bc = (gbuf[:, :, ASRC_ELEM:ASRC_ELEM + 2 // row_b]
                               .bitcast(bf16).broadcast_to([P, C, P]))
                    adt_bc = adT_rep[:].unsqueeze(1).broadcast_to([P, C, P])
                    nc.vector.tensor_tensor(
                        out=w_t[:], in0=asrc_bc, in1=adt_bc,
                        op=mybir.AluOpType.add)
                    wflat = w_t[:].rearrange("p c j -> p (c j)")
                    nc.scalar.activation(
                        out=wflat, in_=wflat,
                        func=mybir.ActivationFunctionType.Prelu,
                        bias=0.0, scale=1.0, alpha=NEG_SLOPE)
                    nc.scalar.activation(
                        out=wflat, in_=wflat,
                        func=mybir.ActivationFunctionType.Exp)

                    # routing matmuls: ps[:, 0:129] += S_w^T @ [z | 1]
                    ps_nd = pbp.tile([P, P + 1], f32, space="PSUM", tag="ps_nd")
                    for c in range(C):
                        S_w = pbs.tile([P, P], bf16, tag="S_w")
                        nc.vector.scalar_tensor_tensor(
                            out=S_w[:], in0=iota_t[:],
                            scalar=dl_t[:, c:c + 1], in1=w_t[:, c, :],
                            op0=mybir.AluOpType.is_equal,
                            op1=mybir.AluOpType.mult)
                        nc.tensor.matmul(ps_nd[:], lhsT=S_w[:],
                                         rhs=gbuf[:, c, 0:P + 1],
                                         start=(c == 0), stop=(c == C - 1))

                    den_s = pbs.tile([P, 1], f32, tag="den_s")
                    nc.vector.tensor_scalar(
                        out=den_s[:], in0=ps_nd[:, P:P + 1], scalar1=1e-30,
                        scalar2=None, op0=mybir.AluOpType.add)
                    den_r = pbs.tile([P, 1], f32, tag="den_r")
                    nc.vector.reciprocal(out=den_r[:], in_=den_s[:])
                    o_t = pbs.tile([P, P], f32, tag="o_t")
                    nc.vector.tensor_scalar(
                        out=o_t[:], in0=ps_nd[:, 0:P], scalar1=den_r[:, 0:1],
                        scalar2=None, op0=mybir.AluOpType.mult)
                    nc.sync.dma_start(
                        out=out_d[k * P:(k + 1) * P, :], in_=o_t[:])

    nc.compile()
    _legalize_waits(nc)
    return nc


# ---------------------------------------------------------------- entry point


def kernel(h, embedding, W_lin, w_fc, w_emb, src, dst):
    global LAST_EXEC_NS
    _apply_tilefix()
    _apply_profhook()
    from concourse import bass_utils

    h = np.asarray(h, dtype=np.float32)
    embedding = np.asarray(embedding, dtype=np.float32)
    W_lin = np.asarray(W_lin, dtype=np.float32)
    w_fc = np.asarray(w_fc, dtype=np.float32).reshape(-1)
    w_emb = np.asarray(w_emb, dtype=np.float32).reshape(-1)

    schedule, per_core = _prepare(src, dst)
    idx_width = per_core[0]["idx"].shape[1]
    key = (schedule, idx_width, ROW_FP8)
    if key not in _BUILD_CACHE:
        _BUILD_CACHE[key] = _build(schedule, idx_width)
    nc = _BUILD_CACHE[key]

    import ml_dtypes
    bf = ml_dtypes.bfloat16
    hT_np = np.zeros((P, NNODES_PAD), bf)
    hT_np[:, :N] = h.T.astype(bf)
    embT_np = np.zeros((P, NNODES_PAD), bf)
    embT_np[:, :N] = embedding.T.astype(bf)
    iota_np = np.tile(np.arange(P)[None, :], (P, 1)).astype(bf)
    ident_np = np.eye(P).astype(bf)
    wfc_np = np.stack([w_fc[:D], w_fc[D:]], axis=1).astype(bf)
    wemb_np = np.stack([w_emb[:D], w_emb[D:]], axis=1).astype(bf)

    in_maps = []
    for m in range(8):
        in_maps.append({
            "hT": hT_np, "embT": embT_np,
            "W_lin": W_lin.astype(bf), "W_linT": W_lin.T.copy().astype(bf),
            "wfc": wfc_np, "wemb": wemb_np,
            "iota_in": iota_np, "ident_in": ident_np,
            "idx_in": per_core[m]["idx"],
            "dstloc_in": per_core[m]["dstloc"],
            "nid_in": per_core[m]["nid"],
        })

    trace = os.environ.get("GAT_TRACE", "0") == "1"
    res = bass_utils.run_bass_kernel_spmd(
        nc, in_maps, core_ids=list(range(8)), trace=trace)
    LAST_EXEC_NS = res.exec_time_ns

    out = np.zeros((NNODES_PAD, P), np.float32)
    for m in range(8):
        t0 = m * TSLOTS
        nt = min(TSLOTS, NTILES - t0)
        if nt <= 0:
            continue
        out[t0 * P:(t0 + nt) * P] = res.results[m]["out"][: nt * P]
    return out[:N]

